# revision 30
# baseline (speedup 1.0000x reference)
"""Trainium2 Bass kernel for GNN message passing (nn_MessageModel).

Reference computation:
    inp = concat([x[col], edge_attr], 1)          # [E, 48]
    h = relu(inp @ W1 + b1)                       # [E, 64]
    messages = h @ W2 + b2                        # [E, 32]
    out = segment_sum(messages, row, N)           # [N, 32]

Strategy (8 NeuronCores, SPMD, destination-node sharding):
- Host: split high-degree nodes into virtual nodes (deg <= CAP), sort
  all virtual nodes by degree (desc), deal them round-robin so each
  core gets an identical degree profile. Nodes are grouped into
  512-slot blocks; block pairs share matmul tiles (2 lanes packed via
  block-diagonal weights). Each (pair, round) is one [96, 512] rhs
  tile holding the r-th edge of each node slot (zero for pad slots).
- HW per (pair, round): W1 matmul [96 -> 128] -> psum; bias+relu
  (alternating Scalar/DVE) -> h bf16; W2 matmul [128 -> 64]
  accumulating rounds into a per-pair PSUM accumulator (start on
  round 0). Segment-sum thus happens in PSUM for free. Drain each
  pair's accumulator to SBUF + DMA out.
- Host: scatter accumulator columns back to node ids, subtract the
  deterministic pad contribution relu(b1) @ W2 per padded round, add
  deg * b2, and merge virtual-node partials.
"""
import sys

if "/opt/trn_rl_repo" not in sys.path:
    sys.path.insert(0, "/opt/trn_rl_repo")

import numpy as np
import ml_dtypes

BF16 = ml_dtypes.bfloat16

N_NODES = 100000
N_EDGES = 1600000
D_NODE = 32
D_EDGE = 16
D_IN = D_NODE + D_EDGE
D_HID = 64
D_OUT = 32

N_CORES = 8
P = 128
GE = 512                 # node slots per block (psum bank cols, f32)
CAP = 14                 # max edges per virtual node (accum chain bound)
B = 4                    # units per input DMA batch
CHUNK = 8                # W1/W2 PE interleave granularity
N_WARM = 24              # PE warmup (pstate ramp) matmuls during DMA startup

_compiled_cache = {}


# ----------------------------------------------------------------------------
# host-side preprocessing
# ----------------------------------------------------------------------------

def _preprocess(x, edge_index, edge_attr, W1, b1, W2, b2):
    x = np.asarray(x, dtype=np.float32)
    W1 = np.asarray(W1, dtype=np.float32)
    W2 = np.asarray(W2, dtype=np.float32)
    b1 = np.asarray(b1, dtype=np.float32)
    b2 = np.asarray(b2, dtype=np.float32)
    row = np.asarray(edge_index[0], dtype=np.int64)
    col = np.asarray(edge_index[1], dtype=np.int64)
    E = row.shape[0]

    order = np.argsort(row, kind="stable")
    col_s = col[order]
    attr_s = np.asarray(edge_attr, dtype=np.float32)[order]
    erow = row[order]

    deg = np.bincount(row, minlength=N_NODES).astype(np.int64)
    cum = np.zeros(N_NODES + 1, dtype=np.int64)
    np.cumsum(deg, out=cum[1:])

    # virtual nodes: chunks of CAP edges
    nv_per = np.maximum(1, -(-deg // CAP))
    vbase = np.zeros(N_NODES + 1, dtype=np.int64)
    np.cumsum(nv_per, out=vbase[1:])
    NV0 = int(vbase[-1])
    vnode_node = np.repeat(np.arange(N_NODES), nv_per)
    vi = np.arange(NV0) - np.repeat(vbase[:-1], nv_per)
    vdeg = np.minimum(deg[vnode_node] - vi * CAP, CAP)

    # sort virtual nodes by degree desc
    vorder = np.argsort(-vdeg, kind="stable")       # vrank -> vnode
    vrank_of = np.empty(NV0, dtype=np.int64)
    vrank_of[vorder] = np.arange(NV0)
    SB = 4096 * 2                                    # ranks per block pair
    NVpad = -(-NV0 // SB) * SB
    vdeg_sorted = np.zeros(NVpad, dtype=np.int64)
    vdeg_sorted[:NV0] = vdeg[vorder]

    npairs_all = NVpad // SB
    R = vdeg_sorted[np.arange(npairs_all) * SB].astype(np.int64)
    npairs = int(np.sum(R > 0))
    R = R[:npairs]
    assert np.all(R[:-1] >= R[1:]) if npairs > 1 else True
    unit_base = np.zeros(npairs + 1, dtype=np.int64)
    np.cumsum(R, out=unit_base[1:])
    U = int(unit_base[-1])
    NB = -(-U // B)

    # per-edge placement
    epos = np.arange(E) - cum[erow]
    ev_i = epos // CAP
    r_e = epos - ev_i * CAP
    evr = vrank_of[vbase[erow] + ev_i]
    j_e = evr // 4096
    w_e = evr % 4096
    s_e = w_e // 8
    k_e = w_e % 8
    lane_e = j_e % 2
    pair_e = j_e // 2
    u_e = unit_base[pair_e] + r_e
    assert np.all(r_e < R[pair_e])

    feats = np.empty((E, D_IN), dtype=BF16)
    feats[:, :D_NODE] = x[col_s]
    feats[:, D_NODE:] = attr_s

    inpT = np.zeros((N_CORES, NB, 2 * D_IN, B * GE), dtype=BF16)
    nb_e = u_e // B
    colpos = (u_e % B) * GE + s_e
    base = ((k_e * NB + nb_e) * (2 * D_IN) + lane_e * D_IN) * (B * GE) + colpos
    idx = base[:, None] + (np.arange(D_IN) * (B * GE))[None, :]
    inpT.reshape(-1)[idx] = feats

    b1_tile = np.tile(b1[:, None], (2, 1))                    # [128, 1]
    W1blk = np.zeros((2 * D_IN, P), dtype=BF16)               # [96, 128]
    W1blk[:D_IN, :D_HID] = W1
    W1blk[D_IN:, D_HID:] = W1
    W2blk = np.zeros((P, 2 * D_OUT), dtype=BF16)              # [128, 64]
    W2blk[:D_HID, :D_OUT] = W2
    W2blk[D_HID:, D_OUT:] = W2

    # pad contribution per padded round: relu(b1) as bf16 through W2
    hpad = np.maximum(b1, 0.0).astype(BF16).astype(np.float32)
    corr = hpad @ W2.astype(BF16).astype(np.float32)          # [32]

    return dict(
        R=R, npairs=npairs, NB=NB, U=U, inpT=inpT,
        b1_tile=b1_tile, W1blk=W1blk, W2blk=W2blk,
        deg=deg, b2=b2, corr=corr,
        vorder=vorder, vdeg=vdeg, vnode_node=vnode_node, NV0=NV0,
    )


# ----------------------------------------------------------------------------
# numpy simulation of the HW dataflow (for correctness debugging)
# ----------------------------------------------------------------------------

def _simulate_hw(prep):
    R, npairs, NB = prep["R"], prep["npairs"], prep["NB"]
    W1f = prep["W1blk"].astype(np.float32)
    W2f = prep["W2blk"].astype(np.float32)
    b1t = prep["b1_tile"][:, 0]
    ext = np.zeros((N_CORES, npairs, 2 * D_OUT, GE), dtype=np.float32)
    for k in range(N_CORES):
        flat = (prep["inpT"][k].astype(np.float32)
                .reshape(NB, 2 * D_IN, B, GE).transpose(0, 2, 1, 3)
                .reshape(NB * B, 2 * D_IN, GE))
        u0 = 0
        for p in range(npairs):
            acc = np.zeros((2 * D_OUT, GE), dtype=np.float32)
            for r in range(R[p]):
                rhs = flat[u0 + r].T                  # [GE, 96] -> use [96, GE]
                hpre = W1f.T @ flat[u0 + r]           # [128, GE]
                h = np.maximum(hpre + b1t[:, None], 0.0).astype(BF16).astype(np.float32)
                acc += W2f.T @ h                       # [64, GE]
            ext[k, p] = acc
            u0 += R[p]
    return ext


# ----------------------------------------------------------------------------
# assembly of the final output
# ----------------------------------------------------------------------------

def _assemble(prep, ext):
    R, npairs = prep["R"], prep["npairs"]
    deg, b2, corr = prep["deg"], prep["b2"], prep["corr"]
    vorder, vdeg, vnode_node, NV0 = (
        prep["vorder"], prep["vdeg"], prep["vnode_node"], prep["NV0"])

    vr = np.arange(NV0)
    vn = vorder                                  # vrank -> vnode idx
    j = vr // 4096
    w = vr % 4096
    s = w // 8
    k = w % 8
    lane = j % 2
    pair = j // 2
    live = pair < npairs

    vals = np.zeros((NV0, D_OUT), dtype=np.float32)
    lv = np.nonzero(live)[0]
    # gather [32] vector for each live vnode
    vals[lv] = ext[k[lv], pair[lv], :, s[lv]].reshape(len(lv), 2, D_OUT)[
        np.arange(len(lv)), lane[lv]]
    npad = np.zeros(NV0, dtype=np.int64)
    npad[lv] = R[pair[lv]] - vdeg[vorder][lv]
    vals -= npad[:, None] * corr[None, :]

    out = np.zeros((N_NODES, D_OUT), dtype=np.float32)
    node_of_vrank = vnode_node[vn]
    np.add.at(out, node_of_vrank, vals)
    out += deg[:, None] * b2[None, :]
    return out


# ----------------------------------------------------------------------------
# bass kernel
# ----------------------------------------------------------------------------

def _build_bass(R, NB):
    import concourse.bacc as bacc
    import concourse.mybir as mybir
    import concourse.tile as tile
    from concourse.tile_rust import add_dep_helper
    from contextlib import ExitStack

    R = list(R)
    npairs = len(R)
    U = sum(R)

    nc = bacc.Bacc("TRN2", target_bir_lowering=False, debug=False,
                   enable_asserts=True, num_devices=N_CORES)
    f32 = mybir.dt.float32
    bf16 = mybir.dt.bfloat16
    inpT_d = nc.dram_tensor("inpT", [NB, 2 * D_IN, B * GE], bf16,
                            kind="ExternalInput").ap()
    W1_d = nc.dram_tensor("W1blk", [2 * D_IN, P], bf16, kind="ExternalInput").ap()
    W2_d = nc.dram_tensor("W2blk", [P, 2 * D_OUT], bf16, kind="ExternalInput").ap()
    b1_d = nc.dram_tensor("b1t", [P, 1], f32, kind="ExternalInput").ap()
    ext_d = nc.dram_tensor("ext", [npairs, 2 * D_OUT, GE], f32,
                           kind="ExternalOutput").ap()

    with tile.TileContext(nc) as tc, ExitStack() as ctx:
        const = ctx.enter_context(tc.tile_pool(name="const", bufs=1))
        sb_in = ctx.enter_context(tc.tile_pool(name="sb_in", bufs=6))
        sb_h = ctx.enter_context(tc.tile_pool(name="sb_h", bufs=4))
        sb_out = ctx.enter_context(tc.tile_pool(name="sb_out", bufs=3))
        ps_h = ctx.enter_context(tc.tile_pool(name="ps_h", bufs=6, space="PSUM"))
        ps_acc = ctx.enter_context(tc.tile_pool(name="ps_acc", bufs=1, space="PSUM"))
        ps_warm = ctx.enter_context(tc.tile_pool(name="ps_warm", bufs=1, space="PSUM"))

        pe_chain = []

        def chain(inst):
            if pe_chain:
                add_dep_helper(inst.ins, pe_chain[-1].ins, sync=False,
                               reason="PE order")
            pe_chain.append(inst)

        # PE warmup: spin matmuls on a zeroed tile so the pstate ramp
        # happens while the first input DMAs are in flight.
        warm_w = const.tile([P, GE], bf16)
        nc.gpsimd.memset(warm_w[:], 0.0)
        warm_p = ps_warm.tile([P, GE], f32, tag="warm")
        for _ in range(N_WARM):
            mm = nc.tensor.matmul(warm_p[:], lhsT=warm_w[:, :P],
                                  rhs=warm_w[:], start=True, stop=True)
            chain(mm)

        # consts on the scalar DGE queue (parallel with input batches on sync)
        W1_s = const.tile([2 * D_IN, P], bf16)
        nc.scalar.dma_start(W1_s[:], W1_d[:])
        W2_s = const.tile([P, 2 * D_OUT], bf16)
        nc.scalar.dma_start(W2_s[:], W2_d[:])
        b1_s = const.tile([P, 1], f32)
        nc.scalar.dma_start(b1_s[:], b1_d[:])

        in_tiles = {}      # batch -> tile
        h_tiles = {}       # pair -> [128, R*GE] tile
        # single psum bank holds two [64, GE] accumulators (alternating pairs)
        acc_bank = ps_acc.tile([P, GE], f32, tag="accbank")

        def acc_slice(p):
            b = (p % 2) * 2 * D_OUT
            return acc_bank[b:b + 2 * D_OUT, :]
        unit_base = [0]
        for r in R:
            unit_base.append(unit_base[-1] + r)

        relu_cnt = [0]

        def rhs_of(u):
            nb, off = divmod(u, B)
            if nb not in in_tiles:
                t = sb_in.tile([2 * D_IN, B * GE], bf16, tag="inp",
                               name=f"in{nb}")
                if nb == 0:
                    for q in range(B):
                        nc.sync.dma_start(t[:, q * GE:(q + 1) * GE],
                                          inpT_d[nb, :, q * GE:(q + 1) * GE])
                else:
                    nc.sync.dma_start(t[:], inpT_d[nb])
                in_tiles[nb] = t
            return in_tiles[nb][:, off * GE:(off + 1) * GE]

        def emit_relu(out_ap, in_ap):
            if relu_cnt[0] % 2 == 0:
                nc.scalar.activation(
                    out=out_ap, in_=in_ap,
                    func=mybir.ActivationFunctionType.Relu, bias=b1_s[:],
                )
            else:
                nc.vector.tensor_scalar(
                    out=out_ap, in0=in_ap, scalar1=b1_s[:], scalar2=0.0,
                    op0=mybir.AluOpType.add, op1=mybir.AluOpType.max,
                )
            relu_cnt[0] += 1

        def emit_w1(p, r):
            if r == 0:
                h_tiles[p] = sb_h.tile([P, CAP * GE], bf16, tag="h",
                                       name=f"h{p}")
            hp = ps_h.tile([P, GE], f32, tag="hpre",
                           name=f"hp{unit_base[p] + r}")
            mm = nc.tensor.matmul(
                hp[:], lhsT=W1_s[:], rhs=rhs_of(unit_base[p] + r),
                start=True, stop=True,
            )
            chain(mm)
            emit_relu(h_tiles[p][:, r * GE:(r + 1) * GE], hp[:])

        def emit_drain(p):
            o = sb_out.tile([2 * D_OUT, GE], f32, tag="ext", name=f"ext{p}")
            nc.scalar.copy(out=o[:], in_=acc_slice(p))
            nc.sync.dma_start(ext_d[p], o[:])
            del h_tiles[p]

        # interleave W1 of pair p with batched W2 of pair p-1
        w2c = [0] * npairs

        def emit_w2_batch(p, n):
            r0 = w2c[p]
            r1 = min(r0 + n, R[p])
            if r1 == r0:
                return
            for r in range(r0, r1):
                mm = nc.tensor.matmul(
                    acc_slice(p), lhsT=W2_s[:],
                    rhs=h_tiles[p][:, r * GE:(r + 1) * GE],
                    start=(r == 0), stop=(r == R[p] - 1),
                    skip_group_check=True,
                )
                chain(mm)
            w2c[p] = r1
            if r1 == R[p]:
                emit_drain(p)

        for p in range(npairs):
            for c0 in range(0, R[p], CHUNK):
                ce = min(c0 + CHUNK, R[p])
                for r in range(c0, ce):
                    emit_w1(p, r)
                if p > 0:
                    emit_w2_batch(p - 1, CHUNK)
            if p > 0:
                emit_w2_batch(p - 1, R[p - 1])
        emit_w2_batch(npairs - 1, R[npairs - 1])

    nc.compile()
    return nc


def _run_hw(prep, trace=False):
    from concourse.bass_utils import run_bass_kernel_spmd

    key = (tuple(prep["R"]), prep["NB"])
    if key not in _compiled_cache:
        _compiled_cache[key] = _build_bass(prep["R"], prep["NB"])
    nc = _compiled_cache[key]

    in_maps = []
    for k in range(N_CORES):
        in_maps.append({
            "inpT": prep["inpT"][k],
            "W1blk": prep["W1blk"],
            "W2blk": prep["W2blk"],
            "b1t": prep["b1_tile"],
        })
    res = run_bass_kernel_spmd(nc, in_maps, list(range(N_CORES)), trace=trace)
    ext = np.stack([res.results[k]["ext"] for k in range(N_CORES)])
    return ext, res


def kernel(x, edge_index, edge_attr, W1, b1, W2, b2, _numpy_sim=False):
    prep = _preprocess(x, edge_index, edge_attr, W1, b1, W2, b2)
    if _numpy_sim:
        ext = _simulate_hw(prep)
    else:
        ext, _ = _run_hw(prep)
    return _assemble(prep, ext)
